# revision 12
# baseline (speedup 1.0000x reference)
"""Trainium2 Bass kernel for nn_Actor_IntentionEncoder (gnn_message_passing).

Data-parallel over the flattened N = B*A = 8192 rows: core c handles rows
[c*1024, (c+1)*1024) = output groups [c*64, (c+1)*64).

Per row n: x = concat(obs[n % 512], latent[n//16, n%16])  [64]
  h   = relu(x @ hw1 + hb1)                               [128]
  w~  = h @ hw2 + hb2     (bias folded into PSUM via K=1 ones matmul)
  emb = tanh(sum_d x[d] * relu(w~[d,:]))                  [128]
  vals = relu(relu(emb@vw1+vb1)@vw2+vb2)
  emb_mean[b] = mean of emb rows [16b,16b+16) -> AllGather (row n pairs with
  emb_mean[n % 512], which spans all cores)
  scores -> exp -> group-sum via PE mask matmuls -> out = (1/S) sum e*vals

Engine split for the hypernet elementwise stage: PE matmuls (bf16) ->
ACT relu-evacuates PSUM->SBUF bf16 -> GPSIMD multiplies by x (per-partition
scalars) -> DVE pairwise add-tree reduces over d -> fp32 accumulator.
"""

import sys

sys.path.insert(0, "/opt/trn_rl_repo")

import ml_dtypes
import numpy as np

import concourse.bacc as bacc
import concourse.bass as bass
import concourse.tile as tile
from concourse import mybir
from concourse.bass_utils import run_bass_kernel_spmd

F32 = mybir.dt.float32
BF16 = mybir.dt.bfloat16
AF = mybir.ActivationFunctionType
NPBF = ml_dtypes.bfloat16

NCORES = 8
B, A, DS, DO, H = 512, 16, 32, 32, 128
D = DS + DO  # 64
N = B * A  # 8192
RPC = N // NCORES  # rows per core = 1024
GPC = B // NCORES  # groups per core = 64
NCHUNK = RPC // 128  # 8 chunks of 128 rows
NSUP = (D * H) // 1024  # 8 supertiles of 1024 cols (8 d-blocks each)

_CACHE = {}


def _build():
    nc = bacc.Bacc("TRN2", target_bir_lowering=False, debug=False,
                   num_devices=NCORES)

    def inp(name, shape, dt=F32):
        return nc.dram_tensor(name, list(shape), dt, kind="ExternalInput").ap()

    xT_d = inp("xT", (D, RPC))
    xrow_d = inp("xrow", (128, NCHUNK * D))      # chunk ch at cols [64ch:64ch+64]
    hw1_d = inp("hw1", (D, H))
    hb1_d = inp("hb1", (H, 1))
    hw2_d = inp("hw2", (H, D * H), BF16)
    b2row_d = inp("b2row", (1, D * H), BF16)
    vw1_d = inp("vw1", (H, H), BF16)
    vb1_d = inp("vb1", (H, 1))
    vw2_d = inp("vw2", (H, H), BF16)
    vb2row_d = inp("vb2row", (1, H), BF16)
    aw1a_d = inp("aw1a", (H, H), BF16)
    aw1b_d = inp("aw1b", (H, H), BF16)
    ab1_d = inp("ab1", (H, 1))
    aw2_d = inp("aw2", (H, H), BF16)
    ab2_d = inp("ab2", (H, 1))
    aw3_d = inp("aw3", (H, 1), BF16)
    ab3_d = inp("ab3", (128, 1))
    onescol_d = inp("onescol", (1, 128), BF16)
    pmask_d = inp("pmask", (128, 8), BF16)
    mavgs_d = inp("mavgs", (128, NCHUNK * GPC))  # chunk ch at cols [64ch:64ch+64]
    ident_d = inp("ident", (128, 128))

    out_d = nc.dram_tensor("out", [GPC, H], F32, kind="ExternalOutput").ap()

    with tile.TileContext(nc) as tc:
        with (
            tc.tile_pool(name="const", bufs=1) as constp,
            tc.tile_pool(name="keep", bufs=1) as keepp,
            tc.tile_pool(name="work", bufs=3) as workp,
            tc.tile_pool(name="acc2", bufs=2) as accp,
            tc.tile_pool(name="wps", bufs=2, space="PSUM") as wpsp,
            tc.tile_pool(name="accps", bufs=1, space="PSUM") as accpsp,
            tc.tile_pool(name="mlpps", bufs=2, space="PSUM") as mlpps,
            tc.tile_pool(name="dram", bufs=1, space="DRAM") as dramp,
        ):
            # ---- load constants to SBUF ----
            def load(ap, shape, name, dt=F32):
                t = constp.tile(list(shape), dt, tag=name, name=name + "_sb")
                nc.sync.dma_start(t[:], ap[:])
                return t

            hw2_sb = constp.tile([H, D * H], BF16, tag="hw2")
            for j in range(8):
                nc.sync.dma_start(hw2_sb[:, j * 1024:(j + 1) * 1024],
                                  hw2_d[:, j * 1024:(j + 1) * 1024])
            xT_sb = load(xT_d, (D, RPC), "xT")
            xrow_sb = load(xrow_d, (128, NCHUNK * D), "xrow")
            hw1_sb = load(hw1_d, (D, H), "hw1")
            hb1_sb = load(hb1_d, (H, 1), "hb1")
            b2row_sb = load(b2row_d, (1, D * H), "b2row", BF16)
            vw1_sb = load(vw1_d, (H, H), "vw1", BF16)
            vb1_sb = load(vb1_d, (H, 1), "vb1")
            vw2_sb = load(vw2_d, (H, H), "vw2", BF16)
            vb2r_sb = load(vb2row_d, (1, H), "vb2r", BF16)
            aw1a_sb = load(aw1a_d, (H, H), "aw1a", BF16)
            aw1b_sb = load(aw1b_d, (H, H), "aw1b", BF16)
            ab1_sb = load(ab1_d, (H, 1), "ab1")
            aw2_sb = load(aw2_d, (H, H), "aw2", BF16)
            ab2_sb = load(ab2_d, (H, 1), "ab2")
            aw3_sb = load(aw3_d, (H, 1), "aw3", BF16)
            ab3_sb = load(ab3_d, (128, 1), "ab3")
            onescol_sb = load(onescol_d, (1, 128), "onescol", BF16)
            pmask_sb = load(pmask_d, (128, 8), "pmask", BF16)
            mavgs_sb = load(mavgs_d, (128, NCHUNK * GPC), "mavgs")
            ident_sb = load(ident_d, (128, 128), "ident")

            embT_sb = keepp.tile([128, NCHUNK * 128], BF16, tag="embT")
            meanT_full = keepp.tile([128, B], BF16, tag="meanTf")

            # ---- h^T = relu(hw1^T @ x^T + hb1)  [128, 1024] bf16 ----
            hT_sb = keepp.tile([H, RPC], BF16, tag="hT")
            hps = wpsp.tile([128, 1024], F32, tag="wps")
            for j in range(2):
                nc.tensor.matmul(hps[:, j * 512:(j + 1) * 512], hw1_sb[:],
                                 xT_sb[:, j * 512:(j + 1) * 512],
                                 start=True, stop=True)
            nc.scalar.activation(hT_sb[:], hps[:], AF.Relu, bias=hb1_sb[:])

            # ---- phase 1: hypernet + emb + emb_mean^T ----
            psum_meanT = accpsp.tile([128, GPC], F32, tag="meanT")
            for ch in range(NCHUNK):
                lhsT_h = hT_sb[:, ch * 128:(ch + 1) * 128]
                emb_acc = accp.tile([128, 128], F32, tag="emb_acc",
                                    name=f"emb_acc_{ch}")
                for sp in range(NSUP // 2):  # supertile pairs: group lhsT reuse
                    wtiles = []
                    for k in range(2):
                        s = sp * 2 + k
                        wps = wpsp.tile([128, 1024], F32, tag="wps",
                                        name=f"wps_{ch}_{s}")
                        for hh in range(2):
                            nc.tensor.matmul(
                                wps[:, hh * 512:(hh + 1) * 512], onescol_sb[:],
                                b2row_sb[:, s * 1024 + hh * 512:
                                         s * 1024 + (hh + 1) * 512],
                                start=True, stop=False)
                        wtiles.append(wps)
                    for k in range(2):
                        s = sp * 2 + k
                        for hh in range(2):
                            nc.tensor.matmul(
                                wtiles[k][:, hh * 512:(hh + 1) * 512], lhsT_h,
                                hw2_sb[:, s * 1024 + hh * 512:
                                       s * 1024 + (hh + 1) * 512],
                                start=False, stop=True)
                    for k in range(2):
                        s = sp * 2 + k
                        wps = wtiles[k]
                        t_bf = workp.tile([128, 1024], BF16, tag="t_bf",
                                          name=f"t_bf_{ch}_{s}")
                        nc.scalar.activation(t_bf[:], wps[:], AF.Relu)
                        y_bf = workp.tile([128, 1024], BF16, tag="y_bf",
                                          name=f"y_bf_{ch}_{s}")
                        for d in range(8):
                            nc.gpsimd.tensor_scalar_mul(
                                y_bf[:, d * 128:(d + 1) * 128],
                                t_bf[:, d * 128:(d + 1) * 128],
                                xrow_sb[:, ch * D + s * 8 + d:
                                        ch * D + s * 8 + d + 1])
                        t1 = workp.tile([128, 512], BF16, tag="t1",
                                        name=f"t1_{ch}_{s}")
                        nc.vector.tensor_add(t1[:], y_bf[:, 0:512],
                                             y_bf[:, 512:1024])
                        t2 = workp.tile([128, 256], BF16, tag="t2",
                                        name=f"t2_{ch}_{s}")
                        nc.vector.tensor_add(t2[:], t1[:, 0:256], t1[:, 256:512])
                        t3 = workp.tile([128, 128], F32, tag="t3",
                                        name=f"t3_{ch}_{s}")
                        nc.vector.tensor_add(t3[:], t2[:, 0:128], t2[:, 128:256])
                        if s == 0:
                            nc.vector.tensor_copy(emb_acc[:], t3[:])
                        else:
                            nc.vector.tensor_add(emb_acc[:], emb_acc[:], t3[:])
                emb_row = workp.tile([128, 128], F32, tag="emb_row",
                                     name=f"emb_row_{ch}")
                nc.scalar.activation(emb_row[:], emb_acc[:], AF.Tanh)
                # emb^T via PE transpose (fp32), evacuated as bf16
                tps = mlpps.tile([128, 128], F32, tag="mlp", name=f"tps_{ch}")
                nc.tensor.transpose(tps[:], emb_row[:], ident_sb[:])
                nc.scalar.copy(embT_sb[:, ch * 128:(ch + 1) * 128], tps[:])
                # emb_mean^T accumulation: [128h, 64g]
                nc.tensor.matmul(psum_meanT[:], emb_row[:],
                                 mavgs_sb[:, ch * GPC:(ch + 1) * GPC],
                                 start=(ch == 0), stop=(ch == NCHUNK - 1))

            # ---- AllGather emb_mean^T (bf16) ----
            meanT_loc = workp.tile([128, GPC], BF16, tag="meanT_loc")
            nc.scalar.copy(meanT_loc[:], psum_meanT[:])
            cc_in = dramp.tile([128, GPC], BF16, tag="cc_in")
            cc_out = dramp.tile([NCORES * 128, GPC], BF16, tag="cc_out")
            nc.sync.dma_start(cc_in[:], meanT_loc[:])
            nc.gpsimd.collective_compute(
                "AllGather", mybir.AluOpType.bypass,
                replica_groups=[list(range(NCORES))],
                ins=[cc_in.opt()], outs=[cc_out.opt()])
            nc.sync.dma_start(
                meanT_full[:].rearrange("p (c g) -> p c g", c=NCORES),
                cc_out[:].rearrange("(c p) g -> p c g", c=NCORES))

            # ---- phase 2: value MLP + attention + weighted output ----
            psum_SO = accpsp.tile([64, 256], F32, tag="SO")
            for ch in range(NCHUNK):
                embT_c = embT_sb[:, ch * 128:(ch + 1) * 128]
                mcol = 128 * (ch % 4)
                v1ps = mlpps.tile([128, 128], F32, tag="mlp", name=f"v1ps_{ch}")
                nc.tensor.matmul(v1ps[:], vw1_sb[:], embT_c, start=True, stop=True)
                v1T = workp.tile([128, 128], BF16, tag="v1T", name=f"v1T_{ch}")
                nc.scalar.activation(v1T[:], v1ps[:], AF.Relu, bias=vb1_sb[:])
                vps = mlpps.tile([128, 128], F32, tag="mlp", name=f"vps_{ch}")
                nc.tensor.matmul(vps[:], onescol_sb[:], vb2r_sb[:],
                                 start=True, stop=False)
                nc.tensor.matmul(vps[:], v1T[:], vw2_sb[:], start=False, stop=True)
                vals = workp.tile([128, 129], BF16, tag="vals", name=f"vals_{ch}")
                nc.scalar.activation(vals[:, 0:128], vps[:], AF.Relu)
                nc.vector.memset(vals[:, 128:129], 1.0)
                y1ps = mlpps.tile([128, 128], F32, tag="mlp", name=f"y1ps_{ch}")
                nc.tensor.matmul(y1ps[:], aw1a_sb[:], embT_c, start=True, stop=False)
                nc.tensor.matmul(y1ps[:], aw1b_sb[:],
                                 meanT_full[:, mcol:mcol + 128],
                                 start=False, stop=True)
                y1T = workp.tile([128, 128], BF16, tag="y1T", name=f"y1T_{ch}")
                nc.scalar.activation(y1T[:], y1ps[:], AF.Relu, bias=ab1_sb[:])
                y2ps = mlpps.tile([128, 128], F32, tag="mlp", name=f"y2ps_{ch}")
                nc.tensor.matmul(y2ps[:], aw2_sb[:], y1T[:], start=True, stop=True)
                y2T = workp.tile([128, 128], BF16, tag="y2T", name=f"y2T_{ch}")
                nc.scalar.activation(y2T[:], y2ps[:], AF.Relu, bias=ab2_sb[:])
                scps = mlpps.tile([128, 128], F32, tag="mlp", name=f"scps_{ch}")
                nc.tensor.matmul(scps[:, 0:1], y2T[:], aw3_sb[:],
                                 start=True, stop=True)
                exp_s = workp.tile([128, 1], F32, tag="exp_s", name=f"exp_s_{ch}")
                nc.scalar.activation(exp_s[:], scps[:, 0:1], AF.Exp,
                                     bias=ab3_sb[:])
                P_w = workp.tile([128, GPC], BF16, tag="P_w", name=f"P_w_{ch}")
                nc.vector.memset(P_w[:], 0.0)
                nc.vector.tensor_scalar_mul(P_w[:, ch * 8:(ch + 1) * 8],
                                            pmask_sb[:], exp_s[:])
                nc.tensor.matmul(psum_SO[:, 0:129], P_w[:], vals[:],
                                 start=(ch == 0), stop=(ch == NCHUNK - 1))

            inv_S = workp.tile([64, 1], F32, tag="inv_S")
            nc.vector.reciprocal(inv_S[:], psum_SO[:, 128:129])
            out_sb = workp.tile([64, 128], F32, tag="out_sb")
            nc.vector.tensor_scalar_mul(out_sb[:], psum_SO[:, 0:128], inv_S[:])
            nc.sync.dma_start(out_d[:], out_sb[:])

    nc.compile()
    return nc


def _prep_inputs(obs, latent, hw1, hb1, hw2, hb2, vw1, vb1, vw2, vb2,
                 aw1, ab1, aw2, ab2, aw3, ab3):
    f = np.float32
    x_full = np.concatenate(
        [np.tile(obs, (A, 1)), latent.reshape(-1, DO)], axis=1).astype(f)  # [N,64]
    pmask = np.zeros((128, 8), NPBF)
    for r in range(128):
        pmask[r, r // 16] = 1.0
    mavgs = np.zeros((128, NCHUNK * GPC), f)
    for ch in range(NCHUNK):
        for r in range(128):
            mavgs[r, ch * GPC + ch * 8 + r // 16] = 1.0 / A
    bf = lambda a: np.ascontiguousarray(np.asarray(a, f).astype(NPBF))
    shared = dict(
        hw1=np.ascontiguousarray(hw1, f),
        hb1=np.ascontiguousarray(np.asarray(hb1, f).reshape(H, 1)),
        hw2=bf(hw2), b2row=bf(np.asarray(hb2).reshape(1, D * H)),
        vw1=bf(vw1), vb1=np.ascontiguousarray(np.asarray(vb1, f).reshape(H, 1)),
        vw2=bf(vw2), vb2row=bf(np.asarray(vb2).reshape(1, H)),
        aw1a=bf(np.asarray(aw1)[:H]), aw1b=bf(np.asarray(aw1)[H:]),
        ab1=np.ascontiguousarray(np.asarray(ab1, f).reshape(H, 1)),
        aw2=bf(aw2),
        ab2=np.ascontiguousarray(np.asarray(ab2, f).reshape(H, 1)),
        aw3=bf(np.asarray(aw3).reshape(H, 1)),
        ab3=np.full((128, 1), np.float32(np.asarray(ab3).reshape(())), f),
        onescol=np.ones((1, 128), NPBF),
        pmask=pmask, mavgs=mavgs, ident=np.eye(128, dtype=f),
    )
    in_maps = []
    for c in range(NCORES):
        xc = x_full[c * RPC:(c + 1) * RPC]  # [1024, 64]
        xrow = np.ascontiguousarray(
            xc.reshape(NCHUNK, 128, D).transpose(1, 0, 2).reshape(128, NCHUNK * D))
        m = dict(shared)
        m["xT"] = np.ascontiguousarray(xc.T)
        m["xrow"] = xrow
        in_maps.append(m)
    return in_maps


def kernel(**inputs):
    obs = np.asarray(inputs["obs"], np.float32)
    latent = np.asarray(inputs["obs_intention_latent"], np.float32)
    in_maps = _prep_inputs(
        obs, latent, inputs["hw1"], inputs["hb1"], inputs["hw2"], inputs["hb2"],
        inputs["vw1"], inputs["vb1"], inputs["vw2"], inputs["vb2"],
        inputs["aw1"], inputs["ab1"], inputs["aw2"], inputs["ab2"],
        inputs["aw3"], inputs["ab3"])
    if "nc" not in _CACHE:
        _CACHE["nc"] = _build()
    res = run_bass_kernel_spmd(_CACHE["nc"], in_maps, list(range(NCORES)))
    _CACHE["res"] = res
    out = np.empty((B, H), np.float32)
    for c in range(NCORES):
        out[c * GPC:(c + 1) * GPC] = res.results[c]["out"]
    return out


if __name__ == "__main__":
    import reference
    inputs = reference.setup_inputs()
    inputs = {k: np.asarray(v) for k, v in inputs.items()}
    got = kernel(**inputs)
    exp = np.asarray(reference.reference(**reference.setup_inputs()))
    print("Relative error:", np.abs(got - exp).max() / (np.abs(exp).max() + 1e-9))
